# revision 37
# baseline (speedup 1.0000x reference)
"""PowerSpectrumModel Trainium2 kernel (8 NeuronCores, SPMD).

Strategy (data-parallel over atoms, structures disjoint per shard):
 - Host: cut the atom axis at structure boundaries into 8 balanced shards;
   quantize ps to fp8-e4m3 with error-feedback dithering over the last
   FB_W feature columns so each atom's psl quantization error cancels
   (no residual stream needed); pre-transpose feature-major and pack two
   512-atom tiles per super-tile so each DMA moves 1 MB contiguously,
   alternating between the Sync- and Pool-engine DMA queues.
 - Device, per 512-atom tile (two-stage software pipeline so no PE
   instruction waits on an activation of its own tile):
     h1    = W1q @ hi   (4 SwInterleave fp8 k-pair matmuls x 2 m-tiles)
     psl   partials: (wp_hi, wp_lo) as M=2 cols -> PSUM rows 0/1
     sil1  = silu(h1/64) -> fp8                                 [ACT]
     h2    = w2q @ sil1 (2 SwInterleave, plain fp8 weights)     [PE]
     sil2  = silu(h2/64 + b2) -> fp8; b2 is a host Gauss-Hermite
             estimate of the systematic w2-quantization error   [ACT]
     psnn  = (wo_hi, wo_lo) M=2 onto psl rows (next tile)       [PE]
     e_row = PSUM rows 0/1 * 1/64 -> fp16 SBUF                  [DVE]
     per 128-chunk: K=2 ones-matmul column-izes partials, + species
     energy (cubic in species number), then a one-hot segment matmul
     into a static 128-struct window (batch is sorted, so each chunk
     spans ~2 structures; +-64 margin covers the CLT deviation). [PE/DVE]
 - Host: slice per-core structure ranges, concat -> [2000, 1].

All weights are replicated, scaled x64 before fp8 quantization (keeps
them out of the e4m3 subnormal range); the single 1/64 unscale rides on
the e_row copy / silu activations. W_psl and W_out use hi+lo fp8 error
compensation; measured rel-err of this scheme is ~8.6e-3 (gate 2e-2).
"""

import numpy as np

N_ATOMS = 200000
N_FEAT = 1024
N_SPECIES = 4
N_STRUCT = 2000
H1 = 256
H2 = 256
SCALE = 1.0
N_CORES = 8
TILE = 512
CHUNK = 128
SMAX = 256  # per-core structure capacity (PSUM row)
SEGW = 64  # one-hot window width per chunk
WS = 64.0  # weight scale before fp8 quantization
FB_W = 128  # feature columns used for psl error-feedback dithering

_BUILD_CACHE = {}
TRACE = False
LAST_EXEC_NS = None
LAST_RESULTS = None


def _split_waits(nc, mybir, maxw=1):
    """walrus on this build rejects >1 sync wait per instruction; move
    overflow waits onto preceding same-engine NoOps."""
    cnt = 0
    for f in nc.m.functions:
        for blk in f.blocks:
            if not hasattr(blk, "instructions"):
                continue
            out = []
            changed = False
            for inst in blk.instructions:
                si = getattr(inst, "sync_info", None)
                if si is not None and si.on_wait and len(si.on_wait) > maxw:
                    waits = list(si.on_wait)
                    keep = waits[-maxw:]
                    extra = waits[:-maxw]
                    while extra:
                        chunk, extra = extra[:maxw], extra[maxw:]
                        cnt += 1
                        out.append(
                            mybir.InstNoOp(
                                name=f"waitfix-{cnt}",
                                engine=inst.engine,
                                text_hint="waitfix",
                                bass_nofuse=True,
                                ins=[],
                                outs=[],
                                sync_info=mybir.SyncInfo(on_wait=chunk, on_update=[]),
                            )
                        )
                    si.on_wait = keep
                    changed = True
                out.append(inst)
            if changed:
                blk.instructions[:] = out
    return cnt


def _build(Ta, C, poly, wins):
    import concourse.bass as bass
    import concourse.tile as tile
    import concourse.mybir as mybir
    from contextlib import ExitStack

    f8 = mybir.dt.float8e4
    f16 = mybir.dt.float16
    f32 = mybir.dt.float32
    AF = mybir.ActivationFunctionType
    ALU = mybir.AluOpType
    DR = mybir.MatmulPerfMode.DoubleRowSwInterleave
    PSUM = bass.MemorySpace.PSUM
    nT = Ta // TILE
    nT2 = nT // 2
    c0, c1, c2, c3 = (float(x) for x in poly)

    nc = bass.Bass("TRN2", target_bir_lowering=False, debug=False)

    ps8_d = nc.dram_tensor(
        "ps8", [nT2, 128, 16 * TILE], f8, kind="ExternalInput"
    ).ap()
    relb_d = nc.dram_tensor("relb", [CHUNK, C], f32, kind="ExternalInput").ap()
    nums_d = nc.dram_tensor("nums", [CHUNK, C], f32, kind="ExternalInput").ap()
    w1_d = nc.dram_tensor("w1", [128, 4 * 2 * 256], f8, kind="ExternalInput").ap()
    wpA_d = nc.dram_tensor("wpA", [128, 4 * 256], f8, kind="ExternalInput").ap()
    w2_d = nc.dram_tensor("w2", [128, 2 * 256], f8, kind="ExternalInput").ap()
    b2_d = nc.dram_tensor("b2", [128, 2], f32, kind="ExternalInput").ap()
    wo_d = nc.dram_tensor("wo", [128, 256], f8, kind="ExternalInput").ap()
    iota_d = nc.dram_tensor("iota", [128, SEGW], f16, kind="ExternalInput").ap()
    out_d = nc.dram_tensor("out", [1, SMAX], f32, kind="ExternalOutput").ap()

    with tile.TileContext(nc) as tc, ExitStack() as ctx:
        const = ctx.enter_context(tc.tile_pool(name="const", bufs=1))
        psTp = ctx.enter_context(tc.tile_pool(name="psT", bufs=6))
        silp = ctx.enter_context(tc.tile_pool(name="sil", bufs=2))
        rowp = ctx.enter_context(tc.tile_pool(name="row", bufs=6))
        pp_h1 = ctx.enter_context(tc.tile_pool(name="pph1", bufs=1, space=PSUM))
        pp_h2 = ctx.enter_context(tc.tile_pool(name="pph2", bufs=1, space=PSUM))
        pp_e = ctx.enter_context(tc.tile_pool(name="ppe", bufs=2, space=PSUM))
        pp_ec = ctx.enter_context(tc.tile_pool(name="ppec", bufs=1, space=PSUM))
        pp_seg = ctx.enter_context(tc.tile_pool(name="ppseg", bufs=1, space=PSUM))

        # ---- constants ----
        w1_sb = const.tile([128, 4, 2, 256], f8, tag="w1")
        nc.gpsimd.dma_start(w1_sb[:], w1_d[:])
        wpA_sb = const.tile([128, 4, 256], f8, tag="wpA")
        nc.gpsimd.dma_start(wpA_sb[:], wpA_d[:])
        w2_sb = const.tile([128, 2, 256], f8, tag="w2")
        nc.gpsimd.dma_start(w2_sb[:], w2_d[:])
        b2_sb = const.tile([128, 2], f32, tag="b2")
        nc.gpsimd.dma_start(b2_sb[:], b2_d[:])
        wo_sb = const.tile([128, 256], f8, tag="wo")
        nc.gpsimd.dma_start(wo_sb[:], wo_d[:])
        iota_sb = const.tile([128, SEGW], f16, tag="iota")
        nc.gpsimd.dma_start(iota_sb[:], iota_d[:])
        relb_sb = const.tile([CHUNK, C], f32, tag="relb")
        nc.gpsimd.dma_start(relb_sb[:], relb_d[:])
        nums_sb = const.tile([CHUNK, C], f32, tag="nums")
        nc.gpsimd.dma_start(nums_sb[:], nums_d[:])
        ones_sb = const.tile([2, 1], f16, tag="ones")
        nc.gpsimd.memset(ones_sb[:], 1.0)

        # species energy per atom: cubic through W_comp[0, 0..3]
        # comp = (c1*n + c0) + n*n*(c3*n + c2)
        t_n2 = const.tile([CHUNK, C], f32, tag="t_n2")
        nc.vector.tensor_mul(t_n2[:], nums_sb[:], nums_sb[:])
        t_a = const.tile([CHUNK, C], f32, tag="t_a")
        nc.vector.tensor_scalar(t_a[:], nums_sb[:], c3, c2, ALU.mult, ALU.add)
        t_b = const.tile([CHUNK, C], f32, tag="t_b")
        nc.vector.tensor_mul(t_b[:], t_n2[:], t_a[:])
        t_c = const.tile([CHUNK, C], f32, tag="t_c")
        nc.vector.tensor_scalar(t_c[:], nums_sb[:], c1, c0, ALU.mult, ALU.add)
        comp_sb = const.tile([CHUNK, C], f32, tag="comp")
        nc.vector.tensor_add(comp_sb[:], t_b[:], t_c[:])

        # seg accumulator: zeroed once; windowed one-hot matmuls accumulate
        # (start=False) into per-chunk [1, SEGW] slices of it.
        seg_ps = pp_seg.tile([1, SMAX], f32, tag="seg")
        nc.vector.memset(seg_ps[:], 0.0)

        def emit_tail_ec(st, ec8, off, start, stop):
            """tile st's deferred column-ize matmuls (read e_row of st)."""
            e_row = st["e_row"]
            for cc in range(4):
                nc.tensor.matmul(
                    ec8[:, off + cc : off + cc + 1],
                    e_row[0:2, cc * 128 : (cc + 1) * 128],
                    ones_sb[:],
                    start=(start and cc == 0),
                    stop=(stop and cc == 3),
                )

        def emit_tail_dve(st, ec8, off):
            """tile st's comp adds (ec8 cols -> fp16 e_col columns)."""
            tt, ecols = st["t"], st["ecols"]
            for cc in range(4):
                ch = tt * 4 + cc
                nc.vector.tensor_add(
                    ecols[cc][:], ec8[:, off + cc : off + cc + 1],
                    comp_sb[:, ch : ch + 1],
                )

        def emit_tail_seg(st):
            """tile st's windowed segment matmuls (emitted after the adds)."""
            tt, ohs, ecols = st["t"], st["ohs"], st["ecols"]
            for cc in range(4):
                ch = tt * 4 + cc
                W = wins[ch]
                nc.tensor.matmul(
                    seg_ps[0:1, W : W + SEGW],
                    ecols[cc][:],
                    ohs[cc][:],
                    start=False,
                    stop=(ch == C - 1),
                    skip_group_check=True,
                )

        # Two-stage software pipeline over tiles:
        #   tile t emits:  h1(t), psl(t) | psnn(t-1) | ec(t-2), adds(t-2),
        #                  seg(t-2) | h2(t) | sil1(t), sil2(t) | e_row(t-1)
        # so no PE instruction ever waits on an activation of its own tile.
        p1 = None  # state awaiting psnn/e_row (tile t-1)
        states = {}  # t -> state awaiting its fp16 tail

        def issue_load(st_i):
            bg = psTp.tile([128, 16, TILE], f8, tag="psT", name=f"psT{st_i}")
            q = nc.sync if st_i % 2 == 0 else nc.gpsimd
            q.dma_start(bg[:], ps8_d[st_i, :, :])
            return bg

        # prefetch two super-tiles ahead so the PE never waits on a load
        bigs = {0: issue_load(0)}
        if nT2 > 1:
            bigs[1] = issue_load(1)
        for t in range(nT):
            st_i, u = divmod(t, 2)
            if u == 0 and st_i + 2 < nT2:
                bigs[st_i + 2] = issue_load(st_i + 2)
            big2 = bigs[st_i]
            big = big2[:, 8 * u : 8 * u + 8, :]

            # ---- h1: SwInterleave k-pairs; sil1 halves start as soon as
            # their m-half of the PSUM closes (separate tiles so the ACT
            # read of m0 does not wait for the m1 matmuls)
            sil1 = silp.tile([128, 2, TILE], f8, tag="sil1")
            h1ps0 = pp_h1.tile([128, TILE], f32, tag="h1m0", name=f"h1ps0_{t}")
            for kp in range(4):
                nc.tensor.matmul(
                    h1ps0[:],
                    w1_sb[:, kp, 0, :],
                    big[:, 2 * kp : 2 * kp + 2, :],
                    start=(kp == 0),
                    stop=(kp == 3),
                    perf_mode=DR,
                )
            nc.scalar.activation(sil1[:, 0, :], h1ps0[:], AF.Silu, scale=1.0 / WS)
            h1ps1 = pp_h1.tile([128, TILE], f32, tag="h1m1", name=f"h1ps1_{t}")
            for kp in range(4):
                nc.tensor.matmul(
                    h1ps1[:],
                    w1_sb[:, kp, 1, :],
                    big[:, 2 * kp : 2 * kp + 2, :],
                    start=(kp == 0),
                    stop=(kp == 3),
                    perf_mode=DR,
                )
            nc.scalar.activation(sil1[:, 1, :], h1ps1[:], AF.Silu, scale=1.0 / WS)

            # ---- tile t-1: psnn (fp8, rides the h1 stream) + e_row
            if p1 is not None:
                nc.tensor.matmul(
                    p1["e_ps"][:],
                    wo_sb[:],
                    p1["sil2"][:],
                    start=False,
                    stop=True,
                    perf_mode=DR,
                )
                e_row = rowp.tile([2, TILE], f16, tag="erow")
                nc.vector.tensor_scalar(
                    e_row[:], p1["e_ps"][0:2, :], 1.0 / WS, None, ALU.mult
                )
                p1["e_row"] = e_row

            # ---- fp16 block only on odd tiles: two tiles' ec (+adds on
            # DVE) and two tiles' seg at once, so the PE pays just one
            # fp8<->fp16 mode-transition pair per two tiles. seg always
            # lags its adds by >=1 tile or a block-internal ~1.4us of ec
            # work, so it never stalls.
            if t % 2 == 1:
                exs = [x for x in (t - 3, t - 2) if x >= 0]
                if exs:
                    ec8 = pp_ec.tile([128, 8], f32, tag="ec", name=f"ec8_{t}")
                    for j, x in enumerate(exs):
                        emit_tail_ec(
                            states[x], ec8, 4 * j, start=(j == 0),
                            stop=(j == len(exs) - 1),
                        )
                    for j, x in enumerate(exs):
                        emit_tail_dve(states[x], ec8, 4 * j)
                for x in (t - 4, t - 3):
                    if x >= 0:
                        emit_tail_seg(states[x])

            # ---- h2: SwInterleave over the 2-chunk sil1 (plain fp8 w2)
            h2ps = pp_h2.tile([128, 2, TILE], f32, tag="h2", name=f"h2ps{t}")
            for m in range(2):
                nc.tensor.matmul(
                    h2ps[:, m, :],
                    w2_sb[:, m, :],
                    sil1[:],
                    start=True,
                    stop=True,
                    perf_mode=DR,
                )
            # silu((h2/64) + b2) per m-chunk: b2 cancels the systematic part
            # of the plain-fp8 w2 quantization error (host Gaussian estimate)
            sil2 = silp.tile([128, 2, TILE], f8, tag="sil2")
            for m in range(2):
                nc.scalar.activation(
                    sil2[:, m, :],
                    h2ps[:, m, :],
                    AF.Silu,
                    bias=b2_sb[:, m : m + 1],
                    scale=1.0 / WS,
                )

            # ---- psl partials LAST: only needs the DMA data and an e_ps
            # buffer that was drained a full tile ago
            e_ps = pp_e.tile([128, TILE], f32, tag="e", name=f"e_ps{t}")
            for kp in range(4):
                nc.tensor.matmul(
                    e_ps[:],
                    wpA_sb[:, kp, :],
                    big[:, 2 * kp : 2 * kp + 2, :],
                    start=(kp == 0),
                    stop=False,
                    perf_mode=DR,
                )

            # ---- one-hot builds for this tile (used by seg two tiles on)
            ohs = []
            for cc in range(4):
                ch = t * 4 + cc
                oh = rowp.tile([128, SEGW], f16, tag=f"oh{cc}")
                nc.vector.tensor_scalar(
                    oh[:], iota_sb[:], relb_sb[:, ch : ch + 1], None, ALU.is_equal
                )
                ohs.append(oh)

            ecols = [
                rowp.tile([128, 1], f16, tag=f"ecol{cc}", name=f"ecol{cc}_{t}")
                for cc in range(4)
            ]
            p1 = {
                "t": t,
                "e_ps": e_ps,
                "sil2": sil2,
                "ohs": ohs,
                "ecols": ecols,
                "e_row": None,
            }
            states[t] = p1

        # ---- drain the pipeline (nT even: pending ec/adds for nT-2 and
        # nT-1, pending seg for nT-3, nT-2, nT-1)
        nc.tensor.matmul(
            p1["e_ps"][:], wo_sb[:], p1["sil2"][:], start=False, stop=True,
            perf_mode=DR,
        )
        e_row = rowp.tile([2, TILE], f16, tag="erow", name="erow_last")
        nc.vector.tensor_scalar(
            e_row[:], p1["e_ps"][0:2, :], 1.0 / WS, None, ALU.mult
        )
        p1["e_row"] = e_row
        ec8f = pp_ec.tile([128, 8], f32, tag="ec", name="ec8_final")
        dxs = [nT - 2, nT - 1]
        for j, x in enumerate(dxs):
            emit_tail_ec(states[x], ec8f, 4 * j, start=(j == 0), stop=(j == 1))
        for j, x in enumerate(dxs):
            emit_tail_dve(states[x], ec8f, 4 * j)
        for x in (nT - 3, nT - 2, nT - 1):
            emit_tail_seg(states[x])

        out_sb = rowp.tile([1, SMAX], f32, tag="outsb")
        nc.scalar.activation(out_sb[:], seg_ps[:], AF.Copy)
        nc.sync.dma_start(out_d[:], out_sb[:])

    _split_waits(nc, mybir)
    return nc


def _install_ntff_hook():
    """Register the axon NTFF profile hook (missing antenv.axon_hooks in
    this image) so run_bass_kernel_spmd(trace=True) can report exec_time_ns."""
    import sys
    import types

    try:
        import antenv.axon_hooks  # noqa: F401

        return
    except ImportError:
        pass
    from trn_agent_boot.trn_boot import _ntff_profile_via_ctypes

    hook = _ntff_profile_via_ctypes("/opt/axon/libaxon_pjrt.so")
    mod = types.ModuleType("antenv.axon_hooks")
    mod.get_axon_ntff_profile_hook = lambda: hook
    mod.set_axon_ntff_profile_hook = lambda h: None
    sys.modules["antenv.axon_hooks"] = mod
    import antenv

    antenv.axon_hooks = mod
    import concourse.bass_utils as bu

    bu.upload_artifacts = lambda tmpdir: tmpdir


def kernel(ps, numbers, batch, W_comp, W_psl, W_h1, W_h2, W_out):
    global LAST_EXEC_NS, LAST_RESULTS
    import ml_dtypes
    from concourse.bass_utils import run_bass_kernel_spmd

    if TRACE:
        _install_ntff_hook()

    f8np = ml_dtypes.float8_e4m3fn

    ps = np.asarray(ps)
    numbers = np.asarray(numbers)
    batch = np.asarray(batch)
    W_comp = np.asarray(W_comp, dtype=np.float32)
    W_psl = np.asarray(W_psl, dtype=np.float32)
    W_h1 = np.asarray(W_h1, dtype=np.float32)
    W_h2 = np.asarray(W_h2, dtype=np.float32)
    W_out = np.asarray(W_out, dtype=np.float32)

    counts = np.bincount(batch, minlength=N_STRUCT)
    cum = np.zeros(N_STRUCT + 1, dtype=np.int64)
    np.cumsum(counts, out=cum[1:])

    # equal-structure shard cuts (atoms balance to ~1-2% by CLT; keeps
    # per-core structure count fixed at N_STRUCT/N_CORES <= SMAX)
    s_cut = [i * N_STRUCT // N_CORES for i in range(N_CORES + 1)]

    shards = []
    for i in range(N_CORES):
        s_lo, s_hi = s_cut[i], s_cut[i + 1]
        a_lo, a_hi = int(cum[s_lo]), int(cum[s_hi])
        n_at, n_st = a_hi - a_lo, s_hi - s_lo
        assert n_st <= SMAX, f"shard {i}: {n_st} structs > {SMAX}"
        shards.append((s_lo, s_hi, a_lo, a_hi, n_at, n_st))

    Ta = max(s[4] for s in shards)
    Ta = (Ta + 2 * TILE - 1) // (2 * TILE) * (2 * TILE)
    nT = Ta // TILE
    nT2 = nT // 2
    C = Ta // CHUNK
    n_st_max = max(s[5] for s in shards)

    # static one-hot windows: chunk ch covers structures near the
    # proportional position; CLT bounds the deviation to a few structs,
    # the +-(SEGW/2) margin is enormous by comparison.
    wins = []
    for ch in range(C):
        est = int(round((ch * CHUNK + CHUNK / 2) * n_st_max / Ta - SEGW / 2))
        wins.append(max(0, min(SMAX - SEGW, est)))

    # fp8 hi with psl error-feedback dithering (no lo residual needed):
    # bulk round-to-nearest, then re-round the last FB_W feature columns
    # so each atom's accumulated psl quantization error cancels.
    w_psl_v = W_psl[0]
    hi_full = ps.astype(f8np)
    c = (hi_full.astype(np.float32) - ps) @ w_psl_v  # [N] accumulated error
    c -= (
        hi_full[:, -FB_W:].astype(np.float32) - ps[:, -FB_W:]
    ) @ w_psl_v[-FB_W:]
    xb = ps[:, -FB_W:]
    _, e_exp = np.frexp(xb)
    ulp = np.ldexp(np.float32(1.0), e_exp - 4).astype(np.float32)
    np.maximum(ulp, np.float32(2.0**-9), out=ulp)
    for j in range(FB_W):
        f = N_FEAT - FB_W + j
        wf = float(w_psl_v[f])
        x = ps[:, f]
        if abs(wf) > 1e-12:
            adj = np.clip(-c / wf, -0.5 * ulp[:, j], 0.5 * ulp[:, j])
            q = (x + adj).astype(f8np)
        else:
            q = x.astype(f8np)
        hi_full[:, f] = q
        c += wf * (q.astype(np.float32) - x)

    # replicated weights, feature-major, x64 scaled fp8 (hi+lo compensation
    # for the accuracy-critical psl row and the small h2/out matrices).
    # DoubleRowSwInterleave weight layout: for a k-tile pair (A, B) each
    # [128, M], the SBUF block is [128, 2M] holding
    # [A_{M-1}, B_{M-1}, ..., A_0, B_0] (pairwise interleave, cols reversed).
    def swi(a, b):
        M = a.shape[1]
        o = np.empty((a.shape[0], 2 * M), dtype=a.dtype)
        o[:, 0::2] = a[:, ::-1]
        o[:, 1::2] = b[:, ::-1]
        return o

    W1s = np.concatenate([W_h1.T, W_psl.T], axis=1) * WS  # [1024, 257]
    w1_q = W1s.astype(f8np)
    w1k = w1_q.reshape(8, 128, 257)  # per k-chunk [128 feat, 257]
    w1 = np.ascontiguousarray(
        np.stack(
            [
                swi(
                    w1k[2 * kp, :, m * 128 : (m + 1) * 128],
                    w1k[2 * kp + 1, :, m * 128 : (m + 1) * 128],
                )
                for kp in range(4)
                for m in range(2)
            ],
            axis=1,
        ).reshape(128, 4 * 2 * 256)
    )
    wp_hi = w1k[:, :, 256]  # [8, 128] fp8 of 64*W_psl per chunk
    wp_lo = (
        W1s[:, 256].reshape(8, 128) - wp_hi.astype(np.float32)
    ).astype(f8np)
    zpad = np.zeros((128, 126), dtype=f8np)

    def pcol(arr, k):  # [8,128] -> [128,1]
        return np.ascontiguousarray(arr[k].reshape(128, 1))

    # psl cols (wp_hi, wp_lo, 0...) padded to M=128 for the SWI
    # active-cols ISA rule. Row 0 collects whi@hi, row 1 wlo@hi.
    wpA = np.ascontiguousarray(
        np.stack(
            [
                swi(
                    np.hstack([pcol(wp_hi, 2 * kp), pcol(wp_lo, 2 * kp), zpad]),
                    np.hstack(
                        [pcol(wp_hi, 2 * kp + 1), pcol(wp_lo, 2 * kp + 1), zpad]
                    ),
                )
                for kp in range(4)
            ],
            axis=1,
        ).reshape(128, 4 * 256)
    )

    W2s = W_h2.T * WS  # [256 in, 256 out]
    w2_q = W2s.astype(f8np)
    w2 = np.ascontiguousarray(
        np.stack(
            [
                swi(
                    w2_q[0:128, m * 128 : (m + 1) * 128],
                    w2_q[128:256, m * 128 : (m + 1) * 128],
                )
                for m in range(2)
            ],
            axis=1,
        ).reshape(128, 2 * 256)
    )
    # systematic part of the w2 quantization error: delta2 @ E[sil1],
    # with E[sil1_j] from the h1 channel stds (weights only, Gauss-Hermite)
    delta2 = (w2_q.astype(np.float32).T - W_h2 * WS) / WS  # [256 out, 256 in]
    w1q_rows = w1_q[:, :256].astype(np.float32).T / WS  # [256, 1024]
    sig1 = np.sqrt((w1q_rows**2).sum(axis=1))
    gh_x, gh_w = np.polynomial.hermite_e.hermegauss(64)
    gh_w = gh_w / gh_w.sum()
    mu1 = np.array(
        [(gh_w * (sig * gh_x / (1 + np.exp(-sig * gh_x)))).sum() for sig in sig1],
        dtype=np.float32,
    )
    bias2 = delta2 @ mu1  # [256]
    b2 = np.ascontiguousarray(-bias2.reshape(2, 128).T.astype(np.float32))

    Wos = W_out[0] * WS  # [256]
    wo_hi = Wos.astype(f8np)
    wo_lo = (Wos - wo_hi.astype(np.float32)).astype(f8np)
    wo = np.ascontiguousarray(
        swi(
            np.hstack([np.stack([wo_hi[0:128], wo_lo[0:128]], axis=1), zpad]),
            np.hstack([np.stack([wo_hi[128:256], wo_lo[128:256]], axis=1), zpad]),
        )
    )

    iota = np.ascontiguousarray(np.tile(np.arange(SEGW, dtype=np.float16), (128, 1)))
    # exact cubic through the 4 species energies
    V = np.vander(np.arange(N_SPECIES, dtype=np.float64), 4, increasing=True)
    poly = np.linalg.solve(V, W_comp[0, :N_SPECIES].astype(np.float64))

    key = (Ta, C, tuple(np.round(poly, 12)), tuple(wins))
    if key not in _BUILD_CACHE:
        _BUILD_CACHE.clear()
        _BUILD_CACHE[key] = _build(Ta, C, poly, wins)
    nc = _BUILD_CACHE[key]

    wins_arr = np.asarray(wins)  # [C]
    in_maps = []
    for s_lo, s_hi, a_lo, a_hi, n_at, n_st in shards:
        hi8 = np.zeros((Ta, N_FEAT), dtype=f8np)
        hi8[:n_at] = hi_full[a_lo:a_hi]
        # pack to [nT2, 128 feat-part, 2 tiles x 8 chunks, 512]
        ps8 = np.ascontiguousarray(
            hi8.reshape(nT2, 2, TILE, 8, 128).transpose(0, 4, 1, 3, 2)
        ).reshape(nT2, 128, 16 * TILE)
        # window-relative struct ids; padding stays negative (no one-hot hit)
        rb = np.full(Ta, -1.0, dtype=np.float32)
        rel = (batch[a_lo:a_hi] - s_lo).astype(np.float32)
        rbw = rel - wins_arr[(np.arange(n_at) // CHUNK)]
        assert (rbw >= 0).all() and (rbw < SEGW).all(), "seg window overflow"
        rb[:n_at] = rbw
        nums = np.zeros(Ta, dtype=np.float32)
        nums[:n_at] = numbers[a_lo:a_hi].astype(np.float32)
        in_maps.append(
            {
                "ps8": ps8,
                "relb": np.ascontiguousarray(rb.reshape(C, CHUNK).T),
                "nums": np.ascontiguousarray(nums.reshape(C, CHUNK).T),
                "w1": w1,
                "wpA": wpA,
                "w2": w2,
                "b2": b2,
                "wo": wo,
                "iota": iota,
            }
        )

    res = run_bass_kernel_spmd(nc, in_maps, list(range(N_CORES)), trace=TRACE)
    LAST_EXEC_NS = res.exec_time_ns
    LAST_RESULTS = res

    out = np.zeros((N_STRUCT, 1), dtype=np.float32)
    for i, (s_lo, s_hi, a_lo, a_hi, n_at, n_st) in enumerate(shards):
        vals = res.results[i]["out"][0, :n_st].astype(np.float32)
        empty = counts[s_lo:s_hi] == 0
        if empty.any():
            vals = np.where(empty, 0.0, vals)
        out[s_lo:s_hi, 0] = vals
    return out


# revision 38
# speedup vs baseline: 1.1039x; 1.1039x over previous
"""PowerSpectrumModel Trainium2 kernel (8 NeuronCores, SPMD).

Strategy (data-parallel over atoms, structures disjoint per shard):
 - Host: cut the atom axis at structure boundaries into 8 balanced shards;
   quantize ps to fp8-e4m3 with error-feedback dithering over the last
   FB_W feature columns so each atom's psl quantization error cancels
   (no residual stream needed); pre-transpose feature-major and pack two
   512-atom tiles per super-tile so each DMA moves 1 MB contiguously,
   alternating between the Sync- and Pool-engine DMA queues.
 - Device, per 512-atom tile (two-stage software pipeline so no PE
   instruction waits on an activation of its own tile):
     h1    = W1q @ hi   (4 SwInterleave fp8 k-pair matmuls x 2 m-tiles)
     psl   partials: (wp_hi, wp_lo) as M=2 cols -> PSUM rows 0/1
     sil1  = silu(h1/64) -> fp8                                 [ACT]
     h2    = w2q @ sil1 (2 SwInterleave, plain fp8 weights)     [PE]
     sil2  = silu(h2/64 + b2) -> fp8; b2 is a host Gauss-Hermite
             estimate of the systematic w2-quantization error   [ACT]
     psnn  = (wo_hi, wo_lo) M=2 onto psl rows (next tile)       [PE]
     e_row = PSUM rows 0/1 * 1/64 -> fp16 SBUF                  [DVE]
     per 128-chunk: K=2 ones-matmul column-izes partials, + species
     energy (cubic in species number), then a one-hot segment matmul
     into a static 128-struct window (batch is sorted, so each chunk
     spans ~2 structures; +-64 margin covers the CLT deviation). [PE/DVE]
 - Host: slice per-core structure ranges, concat -> [2000, 1].

All weights are replicated, scaled x64 before fp8 quantization (keeps
them out of the e4m3 subnormal range); the single 1/64 unscale rides on
the e_row copy / silu activations. W_psl and W_out use hi+lo fp8 error
compensation; measured rel-err of this scheme is ~8.6e-3 (gate 2e-2).
"""

import numpy as np

N_ATOMS = 200000
N_FEAT = 1024
N_SPECIES = 4
N_STRUCT = 2000
H1 = 256
H2 = 256
SCALE = 1.0
N_CORES = 8
TILE = 512
CHUNK = 128
SMAX = 256  # per-core structure capacity (PSUM row)
SEGW = 64  # one-hot window width per chunk
WS = 64.0  # weight scale before fp8 quantization
FB_W = 128  # feature columns used for psl error-feedback dithering

_BUILD_CACHE = {}
TRACE = False
LAST_EXEC_NS = None
LAST_RESULTS = None


def _split_waits(nc, mybir, maxw=1):
    """walrus on this build rejects >1 sync wait per instruction; move
    overflow waits onto preceding same-engine NoOps."""
    cnt = 0
    for f in nc.m.functions:
        for blk in f.blocks:
            if not hasattr(blk, "instructions"):
                continue
            out = []
            changed = False
            for inst in blk.instructions:
                si = getattr(inst, "sync_info", None)
                if si is not None and si.on_wait and len(si.on_wait) > maxw:
                    waits = list(si.on_wait)
                    keep = waits[-maxw:]
                    extra = waits[:-maxw]
                    while extra:
                        chunk, extra = extra[:maxw], extra[maxw:]
                        cnt += 1
                        out.append(
                            mybir.InstNoOp(
                                name=f"waitfix-{cnt}",
                                engine=inst.engine,
                                text_hint="waitfix",
                                bass_nofuse=True,
                                ins=[],
                                outs=[],
                                sync_info=mybir.SyncInfo(on_wait=chunk, on_update=[]),
                            )
                        )
                    si.on_wait = keep
                    changed = True
                out.append(inst)
            if changed:
                blk.instructions[:] = out
    return cnt


def _build(Ta, C, poly, wins):
    import concourse.bass as bass
    import concourse.tile as tile
    import concourse.mybir as mybir
    from contextlib import ExitStack

    f8 = mybir.dt.float8e4
    f16 = mybir.dt.float16
    f32 = mybir.dt.float32
    AF = mybir.ActivationFunctionType
    ALU = mybir.AluOpType
    DR = mybir.MatmulPerfMode.DoubleRowSwInterleave
    PSUM = bass.MemorySpace.PSUM
    nT = Ta // TILE
    nT2 = nT // 2
    c0, c1, c2, c3 = (float(x) for x in poly)

    nc = bass.Bass("TRN2", target_bir_lowering=False, debug=False)

    ps8_d = nc.dram_tensor(
        "ps8", [nT2, 128, 16 * TILE], f8, kind="ExternalInput"
    ).ap()
    relb_d = nc.dram_tensor("relb", [CHUNK, C], f32, kind="ExternalInput").ap()
    nums_d = nc.dram_tensor("nums", [CHUNK, C], f32, kind="ExternalInput").ap()
    w1_d = nc.dram_tensor("w1", [128, 4 * 2 * 256], f8, kind="ExternalInput").ap()
    wpA_d = nc.dram_tensor("wpA", [128, 4 * 256], f8, kind="ExternalInput").ap()
    w2_d = nc.dram_tensor("w2", [128, 2 * 256], f8, kind="ExternalInput").ap()
    b2_d = nc.dram_tensor("b2", [128, 2], f32, kind="ExternalInput").ap()
    wo_d = nc.dram_tensor("wo", [128, 256], f8, kind="ExternalInput").ap()
    iota_d = nc.dram_tensor("iota", [128, SEGW], f16, kind="ExternalInput").ap()
    out_d = nc.dram_tensor("out", [1, SMAX], f32, kind="ExternalOutput").ap()

    with tile.TileContext(nc) as tc, ExitStack() as ctx:
        const = ctx.enter_context(tc.tile_pool(name="const", bufs=1))
        psTp = ctx.enter_context(tc.tile_pool(name="psT", bufs=6))
        silp = ctx.enter_context(tc.tile_pool(name="sil", bufs=2))
        rowp = ctx.enter_context(tc.tile_pool(name="row", bufs=5))
        pp_h1 = ctx.enter_context(tc.tile_pool(name="pph1", bufs=1, space=PSUM))
        pp_h2 = ctx.enter_context(tc.tile_pool(name="pph2", bufs=1, space=PSUM))
        pp_e = ctx.enter_context(tc.tile_pool(name="ppe", bufs=2, space=PSUM))
        pp_ec = ctx.enter_context(tc.tile_pool(name="ppec", bufs=1, space=PSUM))
        pp_seg = ctx.enter_context(tc.tile_pool(name="ppseg", bufs=1, space=PSUM))

        # ---- constants ----
        w1_sb = const.tile([128, 4, 2, 256], f8, tag="w1")
        nc.gpsimd.dma_start(w1_sb[:], w1_d[:])
        wpA_sb = const.tile([128, 4, 256], f8, tag="wpA")
        nc.gpsimd.dma_start(wpA_sb[:], wpA_d[:])
        w2_sb = const.tile([128, 2, 256], f8, tag="w2")
        nc.gpsimd.dma_start(w2_sb[:], w2_d[:])
        b2_sb = const.tile([128, 2], f32, tag="b2")
        nc.gpsimd.dma_start(b2_sb[:], b2_d[:])
        wo_sb = const.tile([128, 256], f8, tag="wo")
        nc.gpsimd.dma_start(wo_sb[:], wo_d[:])
        iota_sb = const.tile([128, SEGW], f16, tag="iota")
        nc.gpsimd.dma_start(iota_sb[:], iota_d[:])
        relb_sb = const.tile([CHUNK, C], f32, tag="relb")
        nc.gpsimd.dma_start(relb_sb[:], relb_d[:])
        nums_sb = const.tile([CHUNK, C], f32, tag="nums")
        nc.gpsimd.dma_start(nums_sb[:], nums_d[:])
        ones_sb = const.tile([2, 1], f16, tag="ones")
        nc.gpsimd.memset(ones_sb[:], 1.0)

        # species energy per atom: cubic through W_comp[0, 0..3]
        # comp = (c1*n + c0) + n*n*(c3*n + c2)
        t_n2 = const.tile([CHUNK, C], f32, tag="t_n2")
        nc.vector.tensor_mul(t_n2[:], nums_sb[:], nums_sb[:])
        t_a = const.tile([CHUNK, C], f32, tag="t_a")
        nc.vector.tensor_scalar(t_a[:], nums_sb[:], c3, c2, ALU.mult, ALU.add)
        t_b = const.tile([CHUNK, C], f32, tag="t_b")
        nc.vector.tensor_mul(t_b[:], t_n2[:], t_a[:])
        t_c = const.tile([CHUNK, C], f32, tag="t_c")
        nc.vector.tensor_scalar(t_c[:], nums_sb[:], c1, c0, ALU.mult, ALU.add)
        comp_sb = const.tile([CHUNK, C], f32, tag="comp")
        nc.vector.tensor_add(comp_sb[:], t_b[:], t_c[:])

        # seg accumulator: zeroed once; windowed one-hot matmuls accumulate
        # (start=False) into per-chunk [1, SEGW] slices of it.
        seg_ps = pp_seg.tile([1, SMAX], f32, tag="seg")
        nc.vector.memset(seg_ps[:], 0.0)

        def emit_tail_ec(st):
            """tile st's deferred column-ize matmuls (read e_row of st)."""
            e_row, ec4 = st["e_row"], st["ec4"]
            for cc in range(4):
                nc.tensor.matmul(
                    ec4[:, cc : cc + 1],
                    e_row[0:2, cc * 128 : (cc + 1) * 128],
                    ones_sb[:],
                    start=(cc == 0),
                    stop=(cc == 3),
                )

        def emit_tail_dve(st):
            """tile st's comp adds (ec4 -> fp16 e_col columns)."""
            tt, ec4, ecols = st["t"], st["ec4"], st["ecols"]
            for cc in range(4):
                ch = tt * 4 + cc
                nc.vector.tensor_add(
                    ecols[cc][:], ec4[:, cc : cc + 1], comp_sb[:, ch : ch + 1]
                )

        def emit_tail_seg(st):
            """tile st's windowed segment matmuls (emitted after the adds)."""
            tt, ohs, ecols = st["t"], st["ohs"], st["ecols"]
            for cc in range(4):
                ch = tt * 4 + cc
                W = wins[ch]
                nc.tensor.matmul(
                    seg_ps[0:1, W : W + SEGW],
                    ecols[cc][:],
                    ohs[cc][:],
                    start=False,
                    stop=(ch == C - 1),
                    skip_group_check=True,
                )

        # Two-stage software pipeline over tiles:
        #   tile t emits:  h1(t), psl(t) | psnn(t-1) | ec(t-2), adds(t-2),
        #                  seg(t-2) | h2(t) | sil1(t), sil2(t) | e_row(t-1)
        # so no PE instruction ever waits on an activation of its own tile.
        p1 = None  # state awaiting psnn/e_row (tile t-1)
        p2 = None  # state awaiting ec/adds (tile t-2)
        p3 = None  # state awaiting seg (tile t-3)

        def issue_load(st_i):
            bg = psTp.tile([128, 16, TILE], f8, tag="psT", name=f"psT{st_i}")
            q = nc.sync if st_i % 2 == 0 else nc.gpsimd
            q.dma_start(bg[:], ps8_d[st_i, :, :])
            return bg

        # prefetch two super-tiles ahead so the PE never waits on a load
        bigs = {0: issue_load(0)}
        if nT2 > 1:
            bigs[1] = issue_load(1)
        for t in range(nT):
            st_i, u = divmod(t, 2)
            if u == 0 and st_i + 2 < nT2:
                bigs[st_i + 2] = issue_load(st_i + 2)
            big2 = bigs[st_i]
            big = big2[:, 8 * u : 8 * u + 8, :]

            # ---- h1: SwInterleave k-pairs; sil1 halves start as soon as
            # their m-half of the PSUM closes (separate tiles so the ACT
            # read of m0 does not wait for the m1 matmuls)
            sil1 = silp.tile([128, 2, TILE], f8, tag="sil1")
            h1ps0 = pp_h1.tile([128, TILE], f32, tag="h1m0", name=f"h1ps0_{t}")
            for kp in range(4):
                nc.tensor.matmul(
                    h1ps0[:],
                    w1_sb[:, kp, 0, :],
                    big[:, 2 * kp : 2 * kp + 2, :],
                    start=(kp == 0),
                    stop=(kp == 3),
                    perf_mode=DR,
                )
            nc.scalar.activation(sil1[:, 0, :], h1ps0[:], AF.Silu, scale=1.0 / WS)
            h1ps1 = pp_h1.tile([128, TILE], f32, tag="h1m1", name=f"h1ps1_{t}")
            for kp in range(4):
                nc.tensor.matmul(
                    h1ps1[:],
                    w1_sb[:, kp, 1, :],
                    big[:, 2 * kp : 2 * kp + 2, :],
                    start=(kp == 0),
                    stop=(kp == 3),
                    perf_mode=DR,
                )
            nc.scalar.activation(sil1[:, 1, :], h1ps1[:], AF.Silu, scale=1.0 / WS)

            # ---- tile t-1: psnn (fp8, rides the h1 stream) + e_row
            if p1 is not None:
                nc.tensor.matmul(
                    p1["e_ps"][:],
                    wo_sb[:],
                    p1["sil2"][:],
                    start=False,
                    stop=True,
                    perf_mode=DR,
                )
                e_row = rowp.tile([2, TILE], f16, tag="erow")
                nc.vector.tensor_scalar(
                    e_row[:], p1["e_ps"][0:2, :], 1.0 / WS, None, ALU.mult
                )
                p1["e_row"] = e_row

            # ---- fp16 block: tile t-2 ec (+adds on DVE), tile t-3 seg.
            # seg lags its adds by a full tile, so it never stalls, and the
            # PE pays only two fp8<->fp16 mode transitions per tile.
            if p2 is not None:
                emit_tail_ec(p2)
                emit_tail_dve(p2)
            if p3 is not None:
                emit_tail_seg(p3)

            # ---- h2: SwInterleave over the 2-chunk sil1 (plain fp8 w2)
            h2ps = pp_h2.tile([128, 2, TILE], f32, tag="h2", name=f"h2ps{t}")
            for m in range(2):
                nc.tensor.matmul(
                    h2ps[:, m, :],
                    w2_sb[:, m, :],
                    sil1[:],
                    start=True,
                    stop=True,
                    perf_mode=DR,
                )
            # silu((h2/64) + b2) per m-chunk: b2 cancels the systematic part
            # of the plain-fp8 w2 quantization error (host Gaussian estimate)
            sil2 = silp.tile([128, 2, TILE], f8, tag="sil2")
            for m in range(2):
                nc.scalar.activation(
                    sil2[:, m, :],
                    h2ps[:, m, :],
                    AF.Silu,
                    bias=b2_sb[:, m : m + 1],
                    scale=1.0 / WS,
                )

            # ---- psl partials LAST: only needs the DMA data and an e_ps
            # buffer that was drained a full tile ago
            e_ps = pp_e.tile([128, TILE], f32, tag="e", name=f"e_ps{t}")
            for kp in range(4):
                nc.tensor.matmul(
                    e_ps[:],
                    wpA_sb[:, kp, :],
                    big[:, 2 * kp : 2 * kp + 2, :],
                    start=(kp == 0),
                    stop=False,
                    perf_mode=DR,
                )

            # ---- one-hot builds for this tile (used by seg two tiles on)
            ohs = []
            for cc in range(4):
                ch = t * 4 + cc
                oh = rowp.tile([128, SEGW], f16, tag=f"oh{cc}")
                nc.vector.tensor_scalar(
                    oh[:], iota_sb[:], relb_sb[:, ch : ch + 1], None, ALU.is_equal
                )
                ohs.append(oh)

            ec4 = pp_ec.tile([128, 4], f32, tag="ec")
            ecols = [
                rowp.tile([128, 1], f16, tag=f"ecol{cc}", name=f"ecol{cc}_{t}")
                for cc in range(4)
            ]
            p3 = p2
            p2 = p1
            p1 = {
                "t": t,
                "e_ps": e_ps,
                "sil2": sil2,
                "ohs": ohs,
                "ec4": ec4,
                "ecols": ecols,
                "e_row": None,
            }

        # ---- drain the pipeline
        nc.tensor.matmul(
            p1["e_ps"][:], wo_sb[:], p1["sil2"][:], start=False, stop=True,
            perf_mode=DR,
        )
        e_row = rowp.tile([2, TILE], f16, tag="erow", name="erow_last")
        nc.vector.tensor_scalar(
            e_row[:], p1["e_ps"][0:2, :], 1.0 / WS, None, ALU.mult
        )
        p1["e_row"] = e_row
        if p2 is not None:
            emit_tail_ec(p2)
            emit_tail_dve(p2)
        if p3 is not None:
            emit_tail_seg(p3)
        emit_tail_ec(p1)
        emit_tail_dve(p1)
        if p2 is not None:
            emit_tail_seg(p2)
        emit_tail_seg(p1)

        out_sb = rowp.tile([1, SMAX], f32, tag="outsb")
        nc.scalar.activation(out_sb[:], seg_ps[:], AF.Copy)
        nc.sync.dma_start(out_d[:], out_sb[:])

    _split_waits(nc, mybir)
    return nc


def _install_ntff_hook():
    """Register the axon NTFF profile hook (missing antenv.axon_hooks in
    this image) so run_bass_kernel_spmd(trace=True) can report exec_time_ns."""
    import sys
    import types

    try:
        import antenv.axon_hooks  # noqa: F401

        return
    except ImportError:
        pass
    from trn_agent_boot.trn_boot import _ntff_profile_via_ctypes

    hook = _ntff_profile_via_ctypes("/opt/axon/libaxon_pjrt.so")
    mod = types.ModuleType("antenv.axon_hooks")
    mod.get_axon_ntff_profile_hook = lambda: hook
    mod.set_axon_ntff_profile_hook = lambda h: None
    sys.modules["antenv.axon_hooks"] = mod
    import antenv

    antenv.axon_hooks = mod
    import concourse.bass_utils as bu

    bu.upload_artifacts = lambda tmpdir: tmpdir


def kernel(ps, numbers, batch, W_comp, W_psl, W_h1, W_h2, W_out):
    global LAST_EXEC_NS, LAST_RESULTS
    import ml_dtypes
    from concourse.bass_utils import run_bass_kernel_spmd

    if TRACE:
        _install_ntff_hook()

    f8np = ml_dtypes.float8_e4m3fn

    ps = np.asarray(ps)
    numbers = np.asarray(numbers)
    batch = np.asarray(batch)
    W_comp = np.asarray(W_comp, dtype=np.float32)
    W_psl = np.asarray(W_psl, dtype=np.float32)
    W_h1 = np.asarray(W_h1, dtype=np.float32)
    W_h2 = np.asarray(W_h2, dtype=np.float32)
    W_out = np.asarray(W_out, dtype=np.float32)

    counts = np.bincount(batch, minlength=N_STRUCT)
    cum = np.zeros(N_STRUCT + 1, dtype=np.int64)
    np.cumsum(counts, out=cum[1:])

    # equal-structure shard cuts (atoms balance to ~1-2% by CLT; keeps
    # per-core structure count fixed at N_STRUCT/N_CORES <= SMAX)
    s_cut = [i * N_STRUCT // N_CORES for i in range(N_CORES + 1)]

    shards = []
    for i in range(N_CORES):
        s_lo, s_hi = s_cut[i], s_cut[i + 1]
        a_lo, a_hi = int(cum[s_lo]), int(cum[s_hi])
        n_at, n_st = a_hi - a_lo, s_hi - s_lo
        assert n_st <= SMAX, f"shard {i}: {n_st} structs > {SMAX}"
        shards.append((s_lo, s_hi, a_lo, a_hi, n_at, n_st))

    Ta = max(s[4] for s in shards)
    Ta = (Ta + 2 * TILE - 1) // (2 * TILE) * (2 * TILE)
    nT = Ta // TILE
    nT2 = nT // 2
    C = Ta // CHUNK
    n_st_max = max(s[5] for s in shards)

    # static one-hot windows: chunk ch covers structures near the
    # proportional position; CLT bounds the deviation to a few structs,
    # the +-(SEGW/2) margin is enormous by comparison.
    wins = []
    for ch in range(C):
        est = int(round((ch * CHUNK + CHUNK / 2) * n_st_max / Ta - SEGW / 2))
        wins.append(max(0, min(SMAX - SEGW, est)))

    # fp8 hi with psl error-feedback dithering (no lo residual needed):
    # bulk round-to-nearest, then re-round the last FB_W feature columns
    # so each atom's accumulated psl quantization error cancels.
    w_psl_v = W_psl[0]
    hi_full = ps.astype(f8np)
    c = (hi_full.astype(np.float32) - ps) @ w_psl_v  # [N] accumulated error
    c -= (
        hi_full[:, -FB_W:].astype(np.float32) - ps[:, -FB_W:]
    ) @ w_psl_v[-FB_W:]
    xb = ps[:, -FB_W:]
    _, e_exp = np.frexp(xb)
    ulp = np.ldexp(np.float32(1.0), e_exp - 4).astype(np.float32)
    np.maximum(ulp, np.float32(2.0**-9), out=ulp)
    for j in range(FB_W):
        f = N_FEAT - FB_W + j
        wf = float(w_psl_v[f])
        x = ps[:, f]
        if abs(wf) > 1e-12:
            adj = np.clip(-c / wf, -0.5 * ulp[:, j], 0.5 * ulp[:, j])
            q = (x + adj).astype(f8np)
        else:
            q = x.astype(f8np)
        hi_full[:, f] = q
        c += wf * (q.astype(np.float32) - x)

    # replicated weights, feature-major, x64 scaled fp8 (hi+lo compensation
    # for the accuracy-critical psl row and the small h2/out matrices).
    # DoubleRowSwInterleave weight layout: for a k-tile pair (A, B) each
    # [128, M], the SBUF block is [128, 2M] holding
    # [A_{M-1}, B_{M-1}, ..., A_0, B_0] (pairwise interleave, cols reversed).
    def swi(a, b):
        M = a.shape[1]
        o = np.empty((a.shape[0], 2 * M), dtype=a.dtype)
        o[:, 0::2] = a[:, ::-1]
        o[:, 1::2] = b[:, ::-1]
        return o

    W1s = np.concatenate([W_h1.T, W_psl.T], axis=1) * WS  # [1024, 257]
    w1_q = W1s.astype(f8np)
    w1k = w1_q.reshape(8, 128, 257)  # per k-chunk [128 feat, 257]
    w1 = np.ascontiguousarray(
        np.stack(
            [
                swi(
                    w1k[2 * kp, :, m * 128 : (m + 1) * 128],
                    w1k[2 * kp + 1, :, m * 128 : (m + 1) * 128],
                )
                for kp in range(4)
                for m in range(2)
            ],
            axis=1,
        ).reshape(128, 4 * 2 * 256)
    )
    wp_hi = w1k[:, :, 256]  # [8, 128] fp8 of 64*W_psl per chunk
    wp_lo = (
        W1s[:, 256].reshape(8, 128) - wp_hi.astype(np.float32)
    ).astype(f8np)
    zpad = np.zeros((128, 126), dtype=f8np)

    def pcol(arr, k):  # [8,128] -> [128,1]
        return np.ascontiguousarray(arr[k].reshape(128, 1))

    # psl cols (wp_hi, wp_lo, 0...) padded to M=128 for the SWI
    # active-cols ISA rule. Row 0 collects whi@hi, row 1 wlo@hi.
    wpA = np.ascontiguousarray(
        np.stack(
            [
                swi(
                    np.hstack([pcol(wp_hi, 2 * kp), pcol(wp_lo, 2 * kp), zpad]),
                    np.hstack(
                        [pcol(wp_hi, 2 * kp + 1), pcol(wp_lo, 2 * kp + 1), zpad]
                    ),
                )
                for kp in range(4)
            ],
            axis=1,
        ).reshape(128, 4 * 256)
    )

    W2s = W_h2.T * WS  # [256 in, 256 out]
    w2_q = W2s.astype(f8np)
    w2 = np.ascontiguousarray(
        np.stack(
            [
                swi(
                    w2_q[0:128, m * 128 : (m + 1) * 128],
                    w2_q[128:256, m * 128 : (m + 1) * 128],
                )
                for m in range(2)
            ],
            axis=1,
        ).reshape(128, 2 * 256)
    )
    # systematic part of the w2 quantization error: delta2 @ E[sil1],
    # with E[sil1_j] from the h1 channel stds (weights only, Gauss-Hermite)
    delta2 = (w2_q.astype(np.float32).T - W_h2 * WS) / WS  # [256 out, 256 in]
    w1q_rows = w1_q[:, :256].astype(np.float32).T / WS  # [256, 1024]
    sig1 = np.sqrt((w1q_rows**2).sum(axis=1))
    gh_x, gh_w = np.polynomial.hermite_e.hermegauss(64)
    gh_w = gh_w / gh_w.sum()
    mu1 = np.array(
        [(gh_w * (sig * gh_x / (1 + np.exp(-sig * gh_x)))).sum() for sig in sig1],
        dtype=np.float32,
    )
    bias2 = delta2 @ mu1  # [256]
    b2 = np.ascontiguousarray(-bias2.reshape(2, 128).T.astype(np.float32))

    Wos = W_out[0] * WS  # [256]
    wo_hi = Wos.astype(f8np)
    wo_lo = (Wos - wo_hi.astype(np.float32)).astype(f8np)
    wo = np.ascontiguousarray(
        swi(
            np.hstack([np.stack([wo_hi[0:128], wo_lo[0:128]], axis=1), zpad]),
            np.hstack([np.stack([wo_hi[128:256], wo_lo[128:256]], axis=1), zpad]),
        )
    )

    iota = np.ascontiguousarray(np.tile(np.arange(SEGW, dtype=np.float16), (128, 1)))
    # exact cubic through the 4 species energies
    V = np.vander(np.arange(N_SPECIES, dtype=np.float64), 4, increasing=True)
    poly = np.linalg.solve(V, W_comp[0, :N_SPECIES].astype(np.float64))

    key = (Ta, C, tuple(np.round(poly, 12)), tuple(wins))
    if key not in _BUILD_CACHE:
        _BUILD_CACHE.clear()
        _BUILD_CACHE[key] = _build(Ta, C, poly, wins)
    nc = _BUILD_CACHE[key]

    wins_arr = np.asarray(wins)  # [C]
    in_maps = []
    for s_lo, s_hi, a_lo, a_hi, n_at, n_st in shards:
        hi8 = np.zeros((Ta, N_FEAT), dtype=f8np)
        hi8[:n_at] = hi_full[a_lo:a_hi]
        # pack to [nT2, 128 feat-part, 2 tiles x 8 chunks, 512]
        ps8 = np.ascontiguousarray(
            hi8.reshape(nT2, 2, TILE, 8, 128).transpose(0, 4, 1, 3, 2)
        ).reshape(nT2, 128, 16 * TILE)
        # window-relative struct ids; padding stays negative (no one-hot hit)
        rb = np.full(Ta, -1.0, dtype=np.float32)
        rel = (batch[a_lo:a_hi] - s_lo).astype(np.float32)
        rbw = rel - wins_arr[(np.arange(n_at) // CHUNK)]
        assert (rbw >= 0).all() and (rbw < SEGW).all(), "seg window overflow"
        rb[:n_at] = rbw
        nums = np.zeros(Ta, dtype=np.float32)
        nums[:n_at] = numbers[a_lo:a_hi].astype(np.float32)
        in_maps.append(
            {
                "ps8": ps8,
                "relb": np.ascontiguousarray(rb.reshape(C, CHUNK).T),
                "nums": np.ascontiguousarray(nums.reshape(C, CHUNK).T),
                "w1": w1,
                "wpA": wpA,
                "w2": w2,
                "b2": b2,
                "wo": wo,
                "iota": iota,
            }
        )

    res = run_bass_kernel_spmd(nc, in_maps, list(range(N_CORES)), trace=TRACE)
    LAST_EXEC_NS = res.exec_time_ns
    LAST_RESULTS = res

    out = np.zeros((N_STRUCT, 1), dtype=np.float32)
    for i, (s_lo, s_hi, a_lo, a_hi, n_at, n_st) in enumerate(shards):
        vals = res.results[i]["out"][0, :n_st].astype(np.float32)
        empty = counts[s_lo:s_hi] == 0
        if empty.any():
            vals = np.where(empty, 0.0, vals)
        out[s_lo:s_hi, 0] = vals
    return out


# revision 39
# speedup vs baseline: 1.1063x; 1.0022x over previous
"""PowerSpectrumModel Trainium2 kernel (8 NeuronCores, SPMD).

Strategy (data-parallel over atoms, structures disjoint per shard):
 - Host: cut the atom axis at structure boundaries into 8 balanced shards;
   quantize ps to fp8-e4m3 with error-feedback dithering over the last
   FB_W feature columns so each atom's psl quantization error cancels
   (no residual stream needed); pre-transpose feature-major and pack two
   512-atom tiles per super-tile so each DMA moves 1 MB contiguously,
   alternating between the Sync- and Pool-engine DMA queues.
 - Device, per 512-atom tile (two-stage software pipeline so no PE
   instruction waits on an activation of its own tile):
     h1    = W1q @ hi   (4 SwInterleave fp8 k-pair matmuls x 2 m-tiles)
     psl   partials: (wp_hi, wp_lo) as M=2 cols -> PSUM rows 0/1
     sil1  = silu(h1/64) -> fp8                                 [ACT]
     h2    = w2q @ sil1 (2 SwInterleave, plain fp8 weights)     [PE]
     sil2  = silu(h2/64 + b2) -> fp8; b2 is a host Gauss-Hermite
             estimate of the systematic w2-quantization error   [ACT]
     psnn  = (wo_hi, wo_lo) M=2 onto psl rows (next tile)       [PE]
     e_row = PSUM rows 0/1 * 1/64 -> fp16 SBUF                  [DVE]
     per 128-chunk: K=2 ones-matmul column-izes partials, + species
     energy (cubic in species number), then a one-hot segment matmul
     into a static 128-struct window (batch is sorted, so each chunk
     spans ~2 structures; +-64 margin covers the CLT deviation). [PE/DVE]
 - Host: slice per-core structure ranges, concat -> [2000, 1].

All weights are replicated, scaled x64 before fp8 quantization (keeps
them out of the e4m3 subnormal range); the single 1/64 unscale rides on
the e_row copy / silu activations. W_psl and W_out use hi+lo fp8 error
compensation; measured rel-err of this scheme is ~8.6e-3 (gate 2e-2).
"""

import numpy as np

N_ATOMS = 200000
N_FEAT = 1024
N_SPECIES = 4
N_STRUCT = 2000
H1 = 256
H2 = 256
SCALE = 1.0
N_CORES = 8
TILE = 512
CHUNK = 128
SMAX = 256  # per-core structure capacity (PSUM row)
SEGW = 32  # one-hot window width per chunk
WS = 64.0  # weight scale before fp8 quantization
FB_W = 128  # feature columns used for psl error-feedback dithering

_BUILD_CACHE = {}
TRACE = False
LAST_EXEC_NS = None
LAST_RESULTS = None


def _split_waits(nc, mybir, maxw=1):
    """walrus on this build rejects >1 sync wait per instruction; move
    overflow waits onto preceding same-engine NoOps."""
    cnt = 0
    for f in nc.m.functions:
        for blk in f.blocks:
            if not hasattr(blk, "instructions"):
                continue
            out = []
            changed = False
            for inst in blk.instructions:
                si = getattr(inst, "sync_info", None)
                if si is not None and si.on_wait and len(si.on_wait) > maxw:
                    waits = list(si.on_wait)
                    keep = waits[-maxw:]
                    extra = waits[:-maxw]
                    while extra:
                        chunk, extra = extra[:maxw], extra[maxw:]
                        cnt += 1
                        out.append(
                            mybir.InstNoOp(
                                name=f"waitfix-{cnt}",
                                engine=inst.engine,
                                text_hint="waitfix",
                                bass_nofuse=True,
                                ins=[],
                                outs=[],
                                sync_info=mybir.SyncInfo(on_wait=chunk, on_update=[]),
                            )
                        )
                    si.on_wait = keep
                    changed = True
                out.append(inst)
            if changed:
                blk.instructions[:] = out
    return cnt


def _build(Ta, C, poly, wins):
    import concourse.bass as bass
    import concourse.tile as tile
    import concourse.mybir as mybir
    from contextlib import ExitStack

    f8 = mybir.dt.float8e4
    f16 = mybir.dt.float16
    f32 = mybir.dt.float32
    AF = mybir.ActivationFunctionType
    ALU = mybir.AluOpType
    DR = mybir.MatmulPerfMode.DoubleRowSwInterleave
    PSUM = bass.MemorySpace.PSUM
    nT = Ta // TILE
    nT2 = nT // 2
    c0, c1, c2, c3 = (float(x) for x in poly)

    nc = bass.Bass("TRN2", target_bir_lowering=False, debug=False)

    ps8_d = nc.dram_tensor(
        "ps8", [nT2, 128, 16 * TILE], f8, kind="ExternalInput"
    ).ap()
    relb_d = nc.dram_tensor("relb", [CHUNK, C], f32, kind="ExternalInput").ap()
    nums_d = nc.dram_tensor("nums", [CHUNK, C], f32, kind="ExternalInput").ap()
    w1_d = nc.dram_tensor("w1", [128, 4 * 2 * 256], f8, kind="ExternalInput").ap()
    wpA_d = nc.dram_tensor("wpA", [128, 4 * 256], f8, kind="ExternalInput").ap()
    w2_d = nc.dram_tensor("w2", [128, 2 * 256], f8, kind="ExternalInput").ap()
    b2_d = nc.dram_tensor("b2", [128, 2], f32, kind="ExternalInput").ap()
    wo_d = nc.dram_tensor("wo", [128, 256], f8, kind="ExternalInput").ap()
    iota_d = nc.dram_tensor("iota", [128, SEGW], f16, kind="ExternalInput").ap()
    out_d = nc.dram_tensor("out", [1, SMAX], f32, kind="ExternalOutput").ap()

    with tile.TileContext(nc) as tc, ExitStack() as ctx:
        const = ctx.enter_context(tc.tile_pool(name="const", bufs=1))
        psTp = ctx.enter_context(tc.tile_pool(name="psT", bufs=6))
        silp = ctx.enter_context(tc.tile_pool(name="sil", bufs=2))
        rowp = ctx.enter_context(tc.tile_pool(name="row", bufs=5))
        pp_h1 = ctx.enter_context(tc.tile_pool(name="pph1", bufs=1, space=PSUM))
        pp_h2 = ctx.enter_context(tc.tile_pool(name="pph2", bufs=1, space=PSUM))
        pp_e = ctx.enter_context(tc.tile_pool(name="ppe", bufs=2, space=PSUM))
        pp_ec = ctx.enter_context(tc.tile_pool(name="ppec", bufs=1, space=PSUM))
        pp_seg = ctx.enter_context(tc.tile_pool(name="ppseg", bufs=1, space=PSUM))

        # ---- constants ----
        w1_sb = const.tile([128, 4, 2, 256], f8, tag="w1")
        nc.gpsimd.dma_start(w1_sb[:], w1_d[:])
        wpA_sb = const.tile([128, 4, 256], f8, tag="wpA")
        nc.gpsimd.dma_start(wpA_sb[:], wpA_d[:])
        w2_sb = const.tile([128, 2, 256], f8, tag="w2")
        nc.gpsimd.dma_start(w2_sb[:], w2_d[:])
        b2_sb = const.tile([128, 2], f32, tag="b2")
        nc.gpsimd.dma_start(b2_sb[:], b2_d[:])
        wo_sb = const.tile([128, 256], f8, tag="wo")
        nc.gpsimd.dma_start(wo_sb[:], wo_d[:])
        iota_sb = const.tile([128, SEGW], f16, tag="iota")
        nc.gpsimd.dma_start(iota_sb[:], iota_d[:])
        relb_sb = const.tile([CHUNK, C], f32, tag="relb")
        nc.gpsimd.dma_start(relb_sb[:], relb_d[:])
        nums_sb = const.tile([CHUNK, C], f32, tag="nums")
        nc.gpsimd.dma_start(nums_sb[:], nums_d[:])
        ones_sb = const.tile([2, 1], f16, tag="ones")
        nc.gpsimd.memset(ones_sb[:], 1.0)

        # species energy per atom: cubic through W_comp[0, 0..3]
        # comp = (c1*n + c0) + n*n*(c3*n + c2)
        t_n2 = const.tile([CHUNK, C], f32, tag="t_n2")
        nc.vector.tensor_mul(t_n2[:], nums_sb[:], nums_sb[:])
        t_a = const.tile([CHUNK, C], f32, tag="t_a")
        nc.vector.tensor_scalar(t_a[:], nums_sb[:], c3, c2, ALU.mult, ALU.add)
        t_b = const.tile([CHUNK, C], f32, tag="t_b")
        nc.vector.tensor_mul(t_b[:], t_n2[:], t_a[:])
        t_c = const.tile([CHUNK, C], f32, tag="t_c")
        nc.vector.tensor_scalar(t_c[:], nums_sb[:], c1, c0, ALU.mult, ALU.add)
        comp_sb = const.tile([CHUNK, C], f32, tag="comp")
        nc.vector.tensor_add(comp_sb[:], t_b[:], t_c[:])

        # seg accumulator: zeroed once; windowed one-hot matmuls accumulate
        # (start=False) into per-chunk [1, SEGW] slices of it.
        seg_ps = pp_seg.tile([1, SMAX], f32, tag="seg")
        nc.vector.memset(seg_ps[:], 0.0)

        def emit_tail_ec(st):
            """tile st's deferred column-ize matmuls (read e_row of st)."""
            e_row, ec4 = st["e_row"], st["ec4"]
            for cc in range(4):
                nc.tensor.matmul(
                    ec4[:, cc : cc + 1],
                    e_row[0:2, cc * 128 : (cc + 1) * 128],
                    ones_sb[:],
                    start=(cc == 0),
                    stop=(cc == 3),
                )

        def emit_tail_dve(st):
            """tile st's comp adds (ec4 -> fp16 e_col columns)."""
            tt, ec4, ecols = st["t"], st["ec4"], st["ecols"]
            for cc in range(4):
                ch = tt * 4 + cc
                nc.vector.tensor_add(
                    ecols[cc][:], ec4[:, cc : cc + 1], comp_sb[:, ch : ch + 1]
                )

        def emit_tail_seg(st):
            """tile st's windowed segment matmuls (emitted after the adds)."""
            tt, ohs, ecols = st["t"], st["ohs"], st["ecols"]
            for cc in range(4):
                ch = tt * 4 + cc
                W = wins[ch]
                nc.tensor.matmul(
                    seg_ps[0:1, W : W + SEGW],
                    ecols[cc][:],
                    ohs[cc][:],
                    start=False,
                    stop=(ch == C - 1),
                    skip_group_check=True,
                )

        # Two-stage software pipeline over tiles:
        #   tile t emits:  h1(t), psl(t) | psnn(t-1) | ec(t-2), adds(t-2),
        #                  seg(t-2) | h2(t) | sil1(t), sil2(t) | e_row(t-1)
        # so no PE instruction ever waits on an activation of its own tile.
        p1 = None  # state awaiting psnn/e_row (tile t-1)
        p2 = None  # state awaiting ec/adds (tile t-2)
        p3 = None  # state awaiting seg (tile t-3)

        def issue_load(st_i):
            bg = psTp.tile([128, 16, TILE], f8, tag="psT", name=f"psT{st_i}")
            q = nc.sync if st_i % 2 == 0 else nc.gpsimd
            q.dma_start(bg[:], ps8_d[st_i, :, :])
            return bg

        # prefetch two super-tiles ahead so the PE never waits on a load
        bigs = {0: issue_load(0)}
        if nT2 > 1:
            bigs[1] = issue_load(1)
        for t in range(nT):
            st_i, u = divmod(t, 2)
            if u == 0 and st_i + 2 < nT2:
                bigs[st_i + 2] = issue_load(st_i + 2)
            big2 = bigs[st_i]
            big = big2[:, 8 * u : 8 * u + 8, :]

            # ---- h1: SwInterleave k-pairs; sil1 halves start as soon as
            # their m-half of the PSUM closes (separate tiles so the ACT
            # read of m0 does not wait for the m1 matmuls)
            sil1 = silp.tile([128, 2, TILE], f8, tag="sil1")
            h1ps0 = pp_h1.tile([128, TILE], f32, tag="h1m0", name=f"h1ps0_{t}")
            for kp in range(4):
                nc.tensor.matmul(
                    h1ps0[:],
                    w1_sb[:, kp, 0, :],
                    big[:, 2 * kp : 2 * kp + 2, :],
                    start=(kp == 0),
                    stop=(kp == 3),
                    perf_mode=DR,
                )
            nc.scalar.activation(sil1[:, 0, :], h1ps0[:], AF.Silu, scale=1.0 / WS)
            h1ps1 = pp_h1.tile([128, TILE], f32, tag="h1m1", name=f"h1ps1_{t}")
            for kp in range(4):
                nc.tensor.matmul(
                    h1ps1[:],
                    w1_sb[:, kp, 1, :],
                    big[:, 2 * kp : 2 * kp + 2, :],
                    start=(kp == 0),
                    stop=(kp == 3),
                    perf_mode=DR,
                )
            nc.scalar.activation(sil1[:, 1, :], h1ps1[:], AF.Silu, scale=1.0 / WS)

            # ---- tile t-1: psnn (fp8, rides the h1 stream) + e_row
            if p1 is not None:
                nc.tensor.matmul(
                    p1["e_ps"][:],
                    wo_sb[:],
                    p1["sil2"][:],
                    start=False,
                    stop=True,
                    perf_mode=DR,
                )
                e_row = rowp.tile([2, TILE], f16, tag="erow")
                nc.vector.tensor_scalar(
                    e_row[:], p1["e_ps"][0:2, :], 1.0 / WS, None, ALU.mult
                )
                p1["e_row"] = e_row

            # ---- fp16 block: tile t-2 ec (+adds on DVE), tile t-3 seg.
            # seg lags its adds by a full tile, so it never stalls, and the
            # PE pays only two fp8<->fp16 mode transitions per tile.
            if p2 is not None:
                emit_tail_ec(p2)
                emit_tail_dve(p2)
            if p3 is not None:
                emit_tail_seg(p3)

            # ---- h2: SwInterleave over the 2-chunk sil1 (plain fp8 w2)
            h2ps = pp_h2.tile([128, 2, TILE], f32, tag="h2", name=f"h2ps{t}")
            for m in range(2):
                nc.tensor.matmul(
                    h2ps[:, m, :],
                    w2_sb[:, m, :],
                    sil1[:],
                    start=True,
                    stop=True,
                    perf_mode=DR,
                )
            # silu((h2/64) + b2) per m-chunk: b2 cancels the systematic part
            # of the plain-fp8 w2 quantization error (host Gaussian estimate)
            sil2 = silp.tile([128, 2, TILE], f8, tag="sil2")
            for m in range(2):
                nc.scalar.activation(
                    sil2[:, m, :],
                    h2ps[:, m, :],
                    AF.Silu,
                    bias=b2_sb[:, m : m + 1],
                    scale=1.0 / WS,
                )

            # ---- psl partials LAST: only needs the DMA data and an e_ps
            # buffer that was drained a full tile ago
            e_ps = pp_e.tile([128, TILE], f32, tag="e", name=f"e_ps{t}")
            for kp in range(4):
                nc.tensor.matmul(
                    e_ps[:],
                    wpA_sb[:, kp, :],
                    big[:, 2 * kp : 2 * kp + 2, :],
                    start=(kp == 0),
                    stop=False,
                    perf_mode=DR,
                )

            # ---- one-hot builds for this tile (used by seg two tiles on)
            ohs = []
            for cc in range(4):
                ch = t * 4 + cc
                oh = rowp.tile([128, SEGW], f16, tag=f"oh{cc}")
                nc.vector.tensor_scalar(
                    oh[:], iota_sb[:], relb_sb[:, ch : ch + 1], None, ALU.is_equal
                )
                ohs.append(oh)

            ec4 = pp_ec.tile([128, 4], f32, tag="ec")
            ecols = [
                rowp.tile([128, 1], f16, tag=f"ecol{cc}", name=f"ecol{cc}_{t}")
                for cc in range(4)
            ]
            p3 = p2
            p2 = p1
            p1 = {
                "t": t,
                "e_ps": e_ps,
                "sil2": sil2,
                "ohs": ohs,
                "ec4": ec4,
                "ecols": ecols,
                "e_row": None,
            }

        # ---- drain the pipeline
        nc.tensor.matmul(
            p1["e_ps"][:], wo_sb[:], p1["sil2"][:], start=False, stop=True,
            perf_mode=DR,
        )
        e_row = rowp.tile([2, TILE], f16, tag="erow", name="erow_last")
        nc.vector.tensor_scalar(
            e_row[:], p1["e_ps"][0:2, :], 1.0 / WS, None, ALU.mult
        )
        p1["e_row"] = e_row
        if p2 is not None:
            emit_tail_ec(p2)
            emit_tail_dve(p2)
        if p3 is not None:
            emit_tail_seg(p3)
        emit_tail_ec(p1)
        emit_tail_dve(p1)
        if p2 is not None:
            emit_tail_seg(p2)
        emit_tail_seg(p1)

        out_sb = rowp.tile([1, SMAX], f32, tag="outsb")
        nc.scalar.activation(out_sb[:], seg_ps[:], AF.Copy)
        nc.sync.dma_start(out_d[:], out_sb[:])

    _split_waits(nc, mybir)
    return nc


def _install_ntff_hook():
    """Register the axon NTFF profile hook (missing antenv.axon_hooks in
    this image) so run_bass_kernel_spmd(trace=True) can report exec_time_ns."""
    import sys
    import types

    try:
        import antenv.axon_hooks  # noqa: F401

        return
    except ImportError:
        pass
    from trn_agent_boot.trn_boot import _ntff_profile_via_ctypes

    hook = _ntff_profile_via_ctypes("/opt/axon/libaxon_pjrt.so")
    mod = types.ModuleType("antenv.axon_hooks")
    mod.get_axon_ntff_profile_hook = lambda: hook
    mod.set_axon_ntff_profile_hook = lambda h: None
    sys.modules["antenv.axon_hooks"] = mod
    import antenv

    antenv.axon_hooks = mod
    import concourse.bass_utils as bu

    bu.upload_artifacts = lambda tmpdir: tmpdir


def kernel(ps, numbers, batch, W_comp, W_psl, W_h1, W_h2, W_out):
    global LAST_EXEC_NS, LAST_RESULTS
    import ml_dtypes
    from concourse.bass_utils import run_bass_kernel_spmd

    if TRACE:
        _install_ntff_hook()

    f8np = ml_dtypes.float8_e4m3fn

    ps = np.asarray(ps)
    numbers = np.asarray(numbers)
    batch = np.asarray(batch)
    W_comp = np.asarray(W_comp, dtype=np.float32)
    W_psl = np.asarray(W_psl, dtype=np.float32)
    W_h1 = np.asarray(W_h1, dtype=np.float32)
    W_h2 = np.asarray(W_h2, dtype=np.float32)
    W_out = np.asarray(W_out, dtype=np.float32)

    counts = np.bincount(batch, minlength=N_STRUCT)
    cum = np.zeros(N_STRUCT + 1, dtype=np.int64)
    np.cumsum(counts, out=cum[1:])

    # equal-structure shard cuts (atoms balance to ~1-2% by CLT; keeps
    # per-core structure count fixed at N_STRUCT/N_CORES <= SMAX)
    s_cut = [i * N_STRUCT // N_CORES for i in range(N_CORES + 1)]

    shards = []
    for i in range(N_CORES):
        s_lo, s_hi = s_cut[i], s_cut[i + 1]
        a_lo, a_hi = int(cum[s_lo]), int(cum[s_hi])
        n_at, n_st = a_hi - a_lo, s_hi - s_lo
        assert n_st <= SMAX, f"shard {i}: {n_st} structs > {SMAX}"
        shards.append((s_lo, s_hi, a_lo, a_hi, n_at, n_st))

    Ta = max(s[4] for s in shards)
    Ta = (Ta + 2 * TILE - 1) // (2 * TILE) * (2 * TILE)
    nT = Ta // TILE
    nT2 = nT // 2
    C = Ta // CHUNK
    n_st_max = max(s[5] for s in shards)

    # static one-hot windows: chunk ch covers structures near the
    # proportional position; CLT bounds the deviation to a few structs,
    # the +-(SEGW/2) margin is enormous by comparison.
    wins = []
    for ch in range(C):
        est = int(round((ch * CHUNK + CHUNK / 2) * n_st_max / Ta - SEGW / 2))
        wins.append(max(0, min(SMAX - SEGW, est)))

    # fp8 hi with psl error-feedback dithering (no lo residual needed):
    # bulk round-to-nearest, then re-round the last FB_W feature columns
    # so each atom's accumulated psl quantization error cancels.
    w_psl_v = W_psl[0]
    hi_full = ps.astype(f8np)
    c = (hi_full.astype(np.float32) - ps) @ w_psl_v  # [N] accumulated error
    c -= (
        hi_full[:, -FB_W:].astype(np.float32) - ps[:, -FB_W:]
    ) @ w_psl_v[-FB_W:]
    xb = ps[:, -FB_W:]
    _, e_exp = np.frexp(xb)
    ulp = np.ldexp(np.float32(1.0), e_exp - 4).astype(np.float32)
    np.maximum(ulp, np.float32(2.0**-9), out=ulp)
    for j in range(FB_W):
        f = N_FEAT - FB_W + j
        wf = float(w_psl_v[f])
        x = ps[:, f]
        if abs(wf) > 1e-12:
            adj = np.clip(-c / wf, -0.5 * ulp[:, j], 0.5 * ulp[:, j])
            q = (x + adj).astype(f8np)
        else:
            q = x.astype(f8np)
        hi_full[:, f] = q
        c += wf * (q.astype(np.float32) - x)

    # replicated weights, feature-major, x64 scaled fp8 (hi+lo compensation
    # for the accuracy-critical psl row and the small h2/out matrices).
    # DoubleRowSwInterleave weight layout: for a k-tile pair (A, B) each
    # [128, M], the SBUF block is [128, 2M] holding
    # [A_{M-1}, B_{M-1}, ..., A_0, B_0] (pairwise interleave, cols reversed).
    def swi(a, b):
        M = a.shape[1]
        o = np.empty((a.shape[0], 2 * M), dtype=a.dtype)
        o[:, 0::2] = a[:, ::-1]
        o[:, 1::2] = b[:, ::-1]
        return o

    W1s = np.concatenate([W_h1.T, W_psl.T], axis=1) * WS  # [1024, 257]
    w1_q = W1s.astype(f8np)
    w1k = w1_q.reshape(8, 128, 257)  # per k-chunk [128 feat, 257]
    w1 = np.ascontiguousarray(
        np.stack(
            [
                swi(
                    w1k[2 * kp, :, m * 128 : (m + 1) * 128],
                    w1k[2 * kp + 1, :, m * 128 : (m + 1) * 128],
                )
                for kp in range(4)
                for m in range(2)
            ],
            axis=1,
        ).reshape(128, 4 * 2 * 256)
    )
    wp_hi = w1k[:, :, 256]  # [8, 128] fp8 of 64*W_psl per chunk
    wp_lo = (
        W1s[:, 256].reshape(8, 128) - wp_hi.astype(np.float32)
    ).astype(f8np)
    zpad = np.zeros((128, 126), dtype=f8np)

    def pcol(arr, k):  # [8,128] -> [128,1]
        return np.ascontiguousarray(arr[k].reshape(128, 1))

    # psl cols (wp_hi, wp_lo, 0...) padded to M=128 for the SWI
    # active-cols ISA rule. Row 0 collects whi@hi, row 1 wlo@hi.
    wpA = np.ascontiguousarray(
        np.stack(
            [
                swi(
                    np.hstack([pcol(wp_hi, 2 * kp), pcol(wp_lo, 2 * kp), zpad]),
                    np.hstack(
                        [pcol(wp_hi, 2 * kp + 1), pcol(wp_lo, 2 * kp + 1), zpad]
                    ),
                )
                for kp in range(4)
            ],
            axis=1,
        ).reshape(128, 4 * 256)
    )

    W2s = W_h2.T * WS  # [256 in, 256 out]
    w2_q = W2s.astype(f8np)
    w2 = np.ascontiguousarray(
        np.stack(
            [
                swi(
                    w2_q[0:128, m * 128 : (m + 1) * 128],
                    w2_q[128:256, m * 128 : (m + 1) * 128],
                )
                for m in range(2)
            ],
            axis=1,
        ).reshape(128, 2 * 256)
    )
    # systematic part of the w2 quantization error: delta2 @ E[sil1],
    # with E[sil1_j] from the h1 channel stds (weights only, Gauss-Hermite)
    delta2 = (w2_q.astype(np.float32).T - W_h2 * WS) / WS  # [256 out, 256 in]
    w1q_rows = w1_q[:, :256].astype(np.float32).T / WS  # [256, 1024]
    sig1 = np.sqrt((w1q_rows**2).sum(axis=1))
    gh_x, gh_w = np.polynomial.hermite_e.hermegauss(64)
    gh_w = gh_w / gh_w.sum()
    mu1 = np.array(
        [(gh_w * (sig * gh_x / (1 + np.exp(-sig * gh_x)))).sum() for sig in sig1],
        dtype=np.float32,
    )
    bias2 = delta2 @ mu1  # [256]
    b2 = np.ascontiguousarray(-bias2.reshape(2, 128).T.astype(np.float32))

    Wos = W_out[0] * WS  # [256]
    wo_hi = Wos.astype(f8np)
    wo_lo = (Wos - wo_hi.astype(np.float32)).astype(f8np)
    wo = np.ascontiguousarray(
        swi(
            np.hstack([np.stack([wo_hi[0:128], wo_lo[0:128]], axis=1), zpad]),
            np.hstack([np.stack([wo_hi[128:256], wo_lo[128:256]], axis=1), zpad]),
        )
    )

    iota = np.ascontiguousarray(np.tile(np.arange(SEGW, dtype=np.float16), (128, 1)))
    # exact cubic through the 4 species energies
    V = np.vander(np.arange(N_SPECIES, dtype=np.float64), 4, increasing=True)
    poly = np.linalg.solve(V, W_comp[0, :N_SPECIES].astype(np.float64))

    key = (Ta, C, tuple(np.round(poly, 12)), tuple(wins))
    if key not in _BUILD_CACHE:
        _BUILD_CACHE.clear()
        _BUILD_CACHE[key] = _build(Ta, C, poly, wins)
    nc = _BUILD_CACHE[key]

    wins_arr = np.asarray(wins)  # [C]
    in_maps = []
    for s_lo, s_hi, a_lo, a_hi, n_at, n_st in shards:
        hi8 = np.zeros((Ta, N_FEAT), dtype=f8np)
        hi8[:n_at] = hi_full[a_lo:a_hi]
        # pack to [nT2, 128 feat-part, 2 tiles x 8 chunks, 512]
        ps8 = np.ascontiguousarray(
            hi8.reshape(nT2, 2, TILE, 8, 128).transpose(0, 4, 1, 3, 2)
        ).reshape(nT2, 128, 16 * TILE)
        # window-relative struct ids; padding stays negative (no one-hot hit)
        rb = np.full(Ta, -1.0, dtype=np.float32)
        rel = (batch[a_lo:a_hi] - s_lo).astype(np.float32)
        rbw = rel - wins_arr[(np.arange(n_at) // CHUNK)]
        assert (rbw >= 0).all() and (rbw < SEGW).all(), "seg window overflow"
        rb[:n_at] = rbw
        nums = np.zeros(Ta, dtype=np.float32)
        nums[:n_at] = numbers[a_lo:a_hi].astype(np.float32)
        in_maps.append(
            {
                "ps8": ps8,
                "relb": np.ascontiguousarray(rb.reshape(C, CHUNK).T),
                "nums": np.ascontiguousarray(nums.reshape(C, CHUNK).T),
                "w1": w1,
                "wpA": wpA,
                "w2": w2,
                "b2": b2,
                "wo": wo,
                "iota": iota,
            }
        )

    res = run_bass_kernel_spmd(nc, in_maps, list(range(N_CORES)), trace=TRACE)
    LAST_EXEC_NS = res.exec_time_ns
    LAST_RESULTS = res

    out = np.zeros((N_STRUCT, 1), dtype=np.float32)
    for i, (s_lo, s_hi, a_lo, a_hi, n_at, n_st) in enumerate(shards):
        vals = res.results[i]["out"][0, :n_st].astype(np.float32)
        empty = counts[s_lo:s_hi] == 0
        if empty.any():
            vals = np.where(empty, 0.0, vals)
        out[s_lo:s_hi, 0] = vals
    return out
